# revision 16
# baseline (speedup 1.0000x reference)
"""Trainium2 Bass kernel for CNN backbone + top-2 MoE head (B=4096).

Data-parallel over 8 NeuronCores (512 images each). All convs run as
single-pass fp16 PE matmuls (fp16 keeps feature error ~3e-4, far under
the 2e-2 gate, at 1 cycle/row):
  conv1: host-built quarter im2col (K=108: 4 row-quarters x 27 taps),
         M=128 (4 quarters x 32 out-ch), columns permuted so each 2x2
         pool window is contiguous -> one DVE pool_max straight from
         PSUM, then a 4x-smaller scalar ACTIVATE (bias+relu->fp16).
  conv2: row bands (K=128: 4 pooled rows x 32 ch), M=128 (2 out-rows x
         64 out-ch), 3 dx passes; pool2 via partition-move DMA +
         gpsimd row-max + DVE pool_max on col pairs.
  conv3: tap-paired K=128 (x+1-shifted copy of the input plane in
         partitions 64:128): 3 pair matmuls + 3 K=64 singles per group.
BN is folded into conv weights/biases host-side. The MoE head keeps the
batch in the matmul free dim (no per-expert transposes) and is
interleaved into the mega loop.
"""
import os
import numpy as np
import ml_dtypes

import concourse.bass as bass
import concourse.mybir as mybir
import concourse.tile as tile
from concourse import bacc
from concourse.bass_utils import run_bass_kernel_spmd
from concourse.masks import make_identity

F32 = mybir.dt.float32
FP16 = mybir.dt.float16

N_CORES = 8
B_FULL = 4096
BC = B_FULL // N_CORES      # 512 images per core
MEGA = 64                   # images per pipeline chunk
NMEGA = BC // MEGA
BN_EPS = 1e-5

f16 = np.float16

_cache = {}
last_result = None


# ---------------------------------------------------------------- host prep

def _fold_bn(w, b, g, beta, mean, var):
    inv = g / np.sqrt(var + BN_EPS)
    wf = w * inv[:, None, None, None]
    bf_ = (b - mean) * inv + beta
    return wf.astype(np.float32), bf_.astype(np.float32)


def _arr1(w):
    """conv1 lhsT [108, 128]: p=(q*27 + c*9 + dy*3 + dx), m=(q*32 + o)."""
    out = np.zeros((108, 128), np.float32)
    for q in range(4):
        for c in range(3):
            for dy in range(3):
                for dx in range(3):
                    out[q * 27 + c * 9 + dy * 3 + dx, q * 32:(q + 1) * 32] = \
                        w[:, c, dy, dx]
    return out


def _arr2(w, dxi):
    """conv2 lhsT [128, 128]: p=(rr*32 + c), m=(yloc*64 + o)."""
    out = np.zeros((128, 128), np.float32)
    for rr in range(4):
        for c in range(32):
            for yloc in range(2):
                dy = rr - yloc
                if 0 <= dy <= 2:
                    out[rr * 32 + c, yloc * 64:(yloc + 1) * 64] = w[:, c, dy, dxi]
    return out


def _build_xq(x):
    """Host-side conv1 quarter im2col -> [108, B, 256] fp16.

    Partition p = q*27 + c*9 + dy*3 + dx. Column n = r*64 + xx*4 +
    ry2*2 + xx2 (pool-window-contiguous order): out pixel
    y = q*8 + 2r + ry2, col = 2*xx + xx2; value = xpad[b, c, y+dy, col+dx]
    with xpad zero-padded by 1.
    """
    n = x.shape[0]
    xpad = np.zeros((n, 3, 34, 34), np.float32)
    xpad[:, :, 1:33, 1:33] = x
    xq = np.zeros((108, n, 256), f16)
    for q in range(4):
        for c in range(3):
            for dy in range(3):
                for dx in range(3):
                    p = q * 27 + c * 9 + dy * 3 + dx
                    sl = xpad[:, c, q * 8 + dy:q * 8 + dy + 8, dx:dx + 32]
                    # [n, 8, 32] -> [n, r(4), ry2(2), xx(16), xx2(2)]
                    v = sl.reshape(n, 4, 2, 16, 2).transpose(0, 1, 3, 2, 4)
                    xq[p] = v.reshape(n, 256).astype(f16)
    return xq


def _prep_weights(inp):
    w1f, b1f = _fold_bn(inp['conv1_w'], inp['conv1_b'], inp['bn1_g'],
                        inp['bn1_b'], inp['bn1_m'], inp['bn1_v'])
    w2f, b2f = _fold_bn(inp['conv2_w'], inp['conv2_b'], inp['bn2_g'],
                        inp['bn2_b'], inp['bn2_m'], inp['bn2_v'])
    w3f, b3f = _fold_bn(inp['conv3_w'], inp['conv3_b'], inp['bn3_g'],
                        inp['bn3_b'], inp['bn3_m'], inp['bn3_v'])

    d = {}
    d['w1p'] = _arr1(w1f).astype(f16)
    d['b1v'] = np.tile(b1f, 4).reshape(128, 1)
    d['w2p'] = np.stack([_arr2(w2f, i) for i in range(3)]).astype(f16)
    d['b2v'] = np.tile(b2f, 2).reshape(128, 1)
    # conv3: pairs (dy, dx=0)+(dy, dx=1) stacked in K; singles (dy, dx=2)
    prs, sgs = [], []
    for dy in range(3):
        prs.append(np.concatenate([w3f[:, :, dy, 0].T, w3f[:, :, dy, 1].T], 0))
        sgs.append(w3f[:, :, dy, 2].T)
    d['w3pr'] = np.ascontiguousarray(np.stack(prs)).astype(f16)   # [3,128,128]
    d['w3sg'] = np.ascontiguousarray(np.stack(sgs)).astype(f16)   # [3,64,128]
    d['b3v'] = b3f.reshape(128, 1)
    # gate / experts (fold the 1/16 avgpool into gate_w and w1)
    d['gw'] = (inp['gate_w'] / 16.0).astype(np.float32)            # [128, 8]
    d['gb'] = inp['gate_b'].reshape(1, 8).astype(np.float32)
    d['w1e'] = np.ascontiguousarray(
        (inp['w1'] / 16.0).transpose(1, 0, 2)).astype(f16)         # [128,8,64]
    d['b1e'] = np.ascontiguousarray(inp['b1'].T).astype(np.float32)  # [64, 8]
    d['w2e'] = np.ascontiguousarray(
        inp['w2'].transpose(1, 0, 2)).astype(f16)                  # [64, 8, 10]
    d['b2e'] = inp['b2'].astype(f16)                               # [8, 10]
    sel = np.zeros((8, 8, 64), np.float32)
    for e in range(8):
        sel[e, e, :] = 1.0
    d['sel8'] = sel.astype(f16)                                    # [8, 8, 64]
    return d


# ---------------------------------------------------------------- device IR

def _build_nc(debug=False):
    nc = bacc.Bacc("TRN2", target_bir_lowering=False, debug=False,
                   enable_asserts=True, num_devices=N_CORES)

    xq_d = nc.dram_tensor("xq", [108, BC, 256], FP16,
                          kind="ExternalInput").ap()
    wd = {}
    for name, shape, dt in [
            ('w1p', [108, 128], FP16), ('b1v', [128, 1], F32),
            ('w2p', [3, 128, 128], FP16), ('b2v', [128, 1], F32),
            ('w3pr', [3, 128, 128], FP16), ('w3sg', [3, 64, 128], FP16),
            ('b3v', [128, 1], F32),
            ('gw', [128, 8], F32), ('gb', [1, 8], F32),
            ('w1e', [128, 8, 64], FP16), ('b1e', [64, 8], F32),
            ('w2e', [64, 8, 10], FP16), ('b2e', [8, 10], FP16),
            ('sel8', [8, 8, 64], FP16)]:
        wd[name] = nc.dram_tensor(name, shape, dt, kind="ExternalInput").ap()
    out_d = nc.dram_tensor("out", [BC, 10], F32, kind="ExternalOutput").ap()
    feat_d = None
    if debug:
        feat_d = nc.dram_tensor("featT", [128, BC], F32, kind="ExternalOutput").ap()

    Relu = mybir.ActivationFunctionType.Relu
    Exp = mybir.ActivationFunctionType.Exp

    with tile.TileContext(nc) as tc:
        with tc.tile_pool(name="persist", bufs=1) as pp, \
             tc.tile_pool(name="xqp", bufs=2) as xqp, \
             tc.tile_pool(name="work", bufs=3) as wp, \
             tc.tile_pool(name="ps", bufs=2, space="PSUM") as psp:

            # --- persistent SBUF tensors
            C1q = pp.tile([128, MEGA, 4, 16], FP16)     # pooled conv1 [q*32+ch]
            bands2h = pp.tile([128, 8, MEGA, 18], FP16)
            c2all = pp.tile([128, 8, MEGA, 16], FP16)
            mv2 = pp.tile([64, 8, MEGA, 16], FP16)
            rm2 = pp.tile([64, 8, MEGA, 16], FP16)
            xp3 = pp.tile([128, MEGA, 10, 10], FP16)
            featT = pp.tile([128, BC], F32)
            ident = pp.tile([128, 128], F32)
            ones_t = pp.tile([1, 128], F32)

            nc.vector.memset(bands2h[:], 0.0)
            nc.vector.memset(xp3[:], 0.0)
            make_identity(nc, ident[:])
            nc.vector.memset(ones_t[:], 1.0)

            # --- weights to SBUF
            ws = {}
            for name, src in wd.items():
                v = src
                if name in ('w2p', 'w3pr', 'w3sg'):
                    v = src.rearrange("d p m -> p d m")
                t = pp.tile(list(v.shape), src.dtype, name="ws_" + name)
                nc.sync.dma_start(out=t[:], in_=v)
                ws[name] = t

            def emit_conv3(m):
                for s3 in range(MEGA // 8):
                    g3 = slice(s3 * 8, (s3 + 1) * 8)
                    ps3 = psp.tile([128, 8, 8, 8], F32, tag="psC")
                    for dy in range(3):
                        nc.tensor.matmul(
                            ps3[:], ws['w3pr'][:, dy, :],
                            xp3[0:128, g3, dy:dy + 8, 0:8],
                            start=(dy == 0), stop=False)
                    for dy in range(3):
                        nc.tensor.matmul(
                            ps3[:], ws['w3sg'][0:64, dy, :],
                            xp3[0:64, g3, dy:dy + 8, 2:10],
                            start=False, stop=(dy == 2))
                    c3o = wp.tile([128, 8, 8, 8], FP16, tag="c3o")
                    nc.scalar.activation(c3o[:], ps3[:], Relu,
                                         bias=ws['b3v'][:], scale=1.0)
                    rm3 = wp.tile([128, 8, 4, 8], FP16, tag="rm3")
                    nc.vector.tensor_max(rm3[:], c3o[:, :, 0::2, :],
                                         c3o[:, :, 1::2, :])
                    cm3 = wp.tile([128, 8, 4, 4], FP16, tag="cm3")
                    nc.vector.tensor_max(cm3[:], rm3[:, :, :, 0::2],
                                         rm3[:, :, :, 1::2])
                    fsl = slice(m * MEGA + s3 * 8, m * MEGA + s3 * 8 + 8)
                    nc.vector.tensor_reduce(
                        featT[:, fsl], cm3[:],
                        axis=mybir.AxisListType.XY, op=mybir.AluOpType.add)

            def emit_head(blk):
                tsl = slice(blk * 128, (blk + 1) * 128)
                lgp = psp.tile([128, 8], F32, tag="psA")
                nc.tensor.matmul(lgp[:], featT[:, tsl], ws['gw'][:],
                                 start=True, stop=False)
                nc.tensor.matmul(lgp[:], ones_t[0:1, :], ws['gb'][:],
                                 start=False, stop=True)
                lg = wp.tile([128, 8], F32, tag="lg")
                nc.scalar.copy(lg[:], lgp[:])
                m1 = wp.tile([128, 1], F32, tag="m1")
                nc.vector.reduce_max(m1[:], lg[:], axis=mybir.AxisListType.X)
                sel1 = wp.tile([128, 8], F32, tag="sel1")
                nc.vector.tensor_scalar(sel1[:], lg[:], m1[:], None,
                                        op0=mybir.AluOpType.is_ge)
                tmp = wp.tile([128, 8], F32, tag="tmp8")
                nc.vector.scalar_tensor_tensor(
                    tmp[:], in0=sel1[:], scalar=-1e30, in1=lg[:],
                    op0=mybir.AluOpType.mult, op1=mybir.AluOpType.add)
                m2 = wp.tile([128, 1], F32, tag="m2")
                nc.vector.reduce_max(m2[:], tmp[:], axis=mybir.AxisListType.X)
                sel = wp.tile([128, 8], F32, tag="sel")
                nc.vector.tensor_scalar(sel[:], lg[:], m2[:], None,
                                        op0=mybir.AluOpType.is_ge)
                negm1 = wp.tile([128, 1], F32, tag="negm1")
                nc.vector.tensor_scalar_mul(negm1[:], m1[:], -1.0)
                ex = wp.tile([128, 8], F32, tag="ex")
                nc.scalar.activation(ex[:], lg[:], Exp, bias=negm1[:], scale=1.0)
                e2 = wp.tile([128, 8], F32, tag="e2")
                nc.vector.tensor_mul(e2[:], ex[:], sel[:])
                ssum = wp.tile([128, 1], F32, tag="ssum")
                nc.vector.reduce_sum(ssum[:], e2[:], axis=mybir.AxisListType.X)
                rcp = wp.tile([128, 1], F32, tag="rcp")
                nc.vector.reciprocal(rcp[:], ssum[:])
                wt = wp.tile([128, 8], F32, tag="wt")
                nc.vector.tensor_scalar(wt[:], e2[:], rcp[:], None,
                                        op0=mybir.AluOpType.mult)
                # wt.T via PE transpose -> fp16 [8, 128]
                wtp = psp.tile([8, 128], F32, tag="psB")
                nc.tensor.transpose(wtp[:], wt[:], ident[0:128, 0:128])
                wtT = wp.tile([8, 128], FP16, tag="wtT")
                nc.scalar.copy(wtT[:], wtp[:])

                fT16 = wp.tile([128, 128], FP16, tag="fT16")
                nc.vector.tensor_copy(fT16[:], featT[:, tsl])

                out_ps = psp.tile([10, 128], F32, tag="psC")
                nc.tensor.matmul(out_ps[:], ws['b2e'][:], wtT[:],
                                 start=True, stop=False)
                for e in range(8):
                    # wtE[64, 128] = broadcast of wtT[e] across 64 partitions
                    wep = psp.tile([64, 128], F32, tag="psB")
                    nc.tensor.matmul(wep[:], ws['sel8'][:, e, :],
                                     wtT[:], start=True, stop=True)
                    wtE = wp.tile([64, 128], FP16, tag="wtE")
                    nc.scalar.copy(wtE[:], wep[:])
                    # heT[64, 128] = relu(w1e_e.T @ featT + b1)
                    hep = psp.tile([64, 128], F32, tag="psA")
                    nc.tensor.matmul(hep[:], ws['w1e'][:, e, :],
                                     fT16[:], start=True, stop=True)
                    he = wp.tile([64, 128], FP16, tag="he")
                    nc.scalar.activation(he[:], hep[:], Relu,
                                         bias=ws['b1e'][:, e:e + 1], scale=1.0)
                    hes = wp.tile([64, 128], FP16, tag="hes")
                    nc.vector.tensor_mul(hes[:], he[:], wtE[:])
                    nc.tensor.matmul(out_ps[:], ws['w2e'][:, e, :], hes[:],
                                     start=False, stop=(e == 7))
                outS = wp.tile([10, 128], F32, tag="outS")
                nc.scalar.copy(outS[:], out_ps[:])
                nc.sync.dma_start(out=out_d[tsl, :].rearrange("b o -> o b"),
                                  in_=outS[:])

            for mega in range(NMEGA):
                g0 = mega * MEGA
                # ---- conv1 im2col: one contiguous DMA per mega
                xq1 = xqp.tile([108, MEGA, 256], FP16, tag="xq1")
                nc.sync.dma_start(out=xq1[:], in_=xq_d[:, g0:g0 + MEGA, :])

                # ---- conv1 matmul + PSUM pool_max + bias/relu evict
                for s in range(MEGA // 2):
                    sl = slice(s * 2, (s + 1) * 2)
                    ps1 = psp.tile([128, 2, 64, 4], F32, tag="psA")
                    nc.tensor.matmul(ps1[:], ws['w1p'][:], xq1[:, sl, :],
                                     start=True, stop=True)
                    pm = wp.tile([128, 2, 64], F32, tag="pm")
                    nc.vector.tensor_reduce(pm[:], ps1[:],
                                            axis=mybir.AxisListType.X,
                                            op=mybir.AluOpType.max)
                    nc.scalar.activation(
                        C1q[:, sl, :, :], pm[:].rearrange("p g (r x) -> p g r x", r=4),
                        Relu, bias=ws['b1v'][:], scale=1.0)

                # ---- conv2 band assembly (batched DMAs)
                # band b2, row-tap rr reads pooled row yp = 2*b2 - 1 + rr
                for rr in range(4):
                    for b2 in range(8):
                        yp = 2 * b2 - 1 + rr
                        if not (0 <= yp < 16):
                            continue
                        q, ry = yp // 4, yp % 4
                        nc.sync.dma_start(
                            out=bands2h[rr * 32:(rr + 1) * 32, b2, :, 1:17],
                            in_=C1q[q * 32:(q + 1) * 32, :, ry, :])

                # ---- conv3 of the previous mega (pipelined)
                if mega > 0:
                    emit_conv3(mega - 1)

                # ---- conv2 (8 bands x 2 half-megas x 3 dx passes)
                for b2 in range(8):
                    for h in range(2):
                        hs = slice(h * 32, (h + 1) * 32)
                        ps2 = psp.tile([128, 32, 16], F32, tag="psB")
                        for dxi in range(3):
                            nc.tensor.matmul(
                                ps2[:], ws['w2p'][:, dxi, :],
                                bands2h[:, b2, hs, dxi:dxi + 16],
                                start=(dxi == 0), stop=(dxi == 2))
                        nc.scalar.activation(c2all[:, b2, hs, :], ps2[:], Relu,
                                             bias=ws['b2v'][:], scale=1.0)
                # ---- pool2: partition move + row max + col pool
                nc.sync.dma_start(out=mv2[:], in_=c2all[64:128, :, :, :])
                nc.vector.tensor_max(rm2[:].rearrange("p a g x -> p (a g x)"),
                                     c2all[0:64, :, :, :].rearrange(
                                         "p a g x -> p (a g x)"),
                                     mv2[:].rearrange("p a g x -> p (a g x)"))
                nc.vector.tensor_max(
                    xp3[0:64, :, 1:9, 1:9].rearrange("p g r x -> p r g x"),
                    rm2[:, :, :, 0::2], rm2[:, :, :, 1::2])
                # shifted copy for conv3 tap pairing: xp3[64:128] = x+1 shift
                for r3 in range(1, 9):
                    nc.sync.dma_start(out=xp3[64:128, :, r3, 0:8],
                                      in_=xp3[0:64, :, r3, 1:9])

                if mega % 2 == 1 and mega > 1:
                    emit_head(mega // 2 - 1)

            # ---- trailing conv3 + head
            emit_conv3(NMEGA - 1)
            if debug:
                nc.sync.dma_start(out=feat_d, in_=featT[:])
            emit_head(NMEGA // 2 - 1)

    nc.compile()
    return nc


# ---------------------------------------------------------------- entry

def kernel(**inputs):
    global last_result
    debug = bool(int(os.environ.get("KERNEL_DEBUG", "0")))
    key = ("nc", debug)
    if key not in _cache:
        _cache[key] = _build_nc(debug=debug)
    nc = _cache[key]

    w = _prep_weights(inputs)
    x = np.asarray(inputs['x'], np.float32)
    xq = _build_xq(x)  # [108, B, 256] fp16

    in_maps = []
    for c in range(N_CORES):
        sl = slice(c * BC, (c + 1) * BC)
        m = {'xq': np.ascontiguousarray(xq[:, sl])}
        for k, v in w.items():
            m[k] = v
        in_maps.append(m)

    trace = bool(int(os.environ.get("KERNEL_TRACE", "0")))
    res = run_bass_kernel_spmd(nc, in_maps, core_ids=list(range(N_CORES)),
                               trace=trace)
    last_result = res
    out = np.concatenate([res.results[c]["out"] for c in range(N_CORES)], 0)
    return out.astype(np.float32)
